# revision 4
# baseline (speedup 1.0000x reference)
"""Trainium2 Bass kernel for nn_HODE_MDP (hypergraph ODE message passing).

Math (T_UP = T_GEO = T_P2P = 1.0, ALPHA = 0.8):
    pe  = poi_emb_weight[:-1]                      # [P, D]
    x/s/g = pe * sigmoid(pe @ W_t + b_t)           # col / seq / geo gates
    hg_pois    = x + HG_pu @ (HG_up @ x)
    geo_pois   = g + 0.4 * (poi_geo_graph @ g)
    trans_pois = s + HG_poi_src @ (HG_poi_tar @ s)
    hg_users   = (HG_up @ hg_pois)[user_idx]
    geo_users  = (HG_up @ geo_pois)[user_idx]
    out = concat([hg_pois, geo_pois, trans_pois, hg_users, geo_users])

Distribution (8 NeuronCores): shard the row dim of every big matrix
(P rows for HG_pu / HG_poi_src / poi_geo_graph, U rows for HG_up, E rows
for HG_poi_tar).  Each core computes its row-block of each product with
the full activation (all-gathered [*, D] tensors).  Matrix blocks are
shipped pre-transposed ([contract_dim, out_block]) so the PE consumes
them as the moving operand with N=512; the activation k-tile [128, 128]
is the stationary operand.  All products run in bf16 (f32 PSUM accum);
gates stay f32 — the ODE deltas are ~1e-5 of the output scale so bf16
matmul error is negligible (measured l2 rel err 6.6e-6 on the full
pipeline vs f32).

Outputs come back transposed [D, block]; the host re-transposes,
concatenates, and applies the user_idx gather.
"""

import sys

if "/opt/trn_rl_repo" not in sys.path:
    sys.path.insert(0, "/opt/trn_rl_repo")

import numpy as np
import ml_dtypes

import concourse.bass as bass  # noqa: F401  (engine types via nc)
import concourse.bacc as bacc
import concourse.mybir as mybir
import concourse.tile as tile
from concourse.bass_utils import run_bass_kernel_spmd

F32 = mybir.dt.float32
BF16 = mybir.dt.bfloat16
SIG = mybir.ActivationFunctionType.Sigmoid
MULT = mybir.AluOpType.mult
ADD = mybir.AluOpType.add
BYPASS = mybir.AluOpType.bypass

NCORES = 8
P, U, E, D = 8192, 4096, 4096, 128
PP, UU, EE = P // NCORES, U // NCORES, E // NCORES  # 1024, 512, 512
KP, KU = P // 128, U // 128                         # 64, 32 k-tiles
RG = [list(range(NCORES))]
GEO_SCALE = 0.4  # ALPHA / 2 * T_GEO

_CACHE: dict = {}


def _build_nc():
    nc = bacc.Bacc(
        "TRN2",
        target_bir_lowering=False,
        debug=False,
        enable_asserts=False,
        num_devices=NCORES,
    )

    # ---- per-core DRAM I/O ----------------------------------------------
    peT = nc.dram_tensor("peT", [D, PP], F32, kind="ExternalInput").ap()
    w3 = nc.dram_tensor("w3", [3, D, D], F32, kind="ExternalInput").ap()
    bT3 = nc.dram_tensor("bT3", [D, 3], F32, kind="ExternalInput").ap()
    ident = nc.dram_tensor("ident", [D, D], F32, kind="ExternalInput").ap()
    UpT = nc.dram_tensor("UpT", [P, UU], BF16, kind="ExternalInput").ap()
    PuT = nc.dram_tensor("PuT", [U, PP], BF16, kind="ExternalInput").ap()
    TarT = nc.dram_tensor("TarT", [P, EE], BF16, kind="ExternalInput").ap()
    SrcT = nc.dram_tensor("SrcT", [E, PP], BF16, kind="ExternalInput").ap()
    GeoT = nc.dram_tensor("GeoT", [P, PP], BF16, kind="ExternalInput").ap()

    poisT_o = nc.dram_tensor("poisT_o", [3, D, PP], F32, kind="ExternalOutput").ap()
    usersT_o = nc.dram_tensor("usersT_o", [D, 2 * UU], F32, kind="ExternalOutput").ap()

    with tile.TileContext(nc) as tc:
        with (
            tc.tile_pool(name="const", bufs=1) as constp,
            tc.tile_pool(name="big", bufs=3) as bigp,
            tc.tile_pool(name="rhs", bufs=6) as rhsp,
            tc.tile_pool(name="stage", bufs=2) as stagep,
            tc.tile_pool(name="outp", bufs=2) as outp,
            tc.tile_pool(name="psacc", bufs=2, space="PSUM") as psacc,
            tc.tile_pool(name="pstr", bufs=2, space="PSUM") as pstr,
            tc.tile_pool(name="dram", bufs=1, space="DRAM") as dramp,
        ):
            # ---- internal DRAM collective buffers -----------------------
            cc_gates_in = dramp.tile([3 * PP, D], BF16, name="cc_gates_in")
            cc_gates_out = dramp.tile(
                [NCORES * 3 * PP, D], BF16, addr_space="Shared", name="cc_gates_out"
            )
            cc_y_in = dramp.tile([UU + EE, D], BF16, name="cc_y_in")
            cc_y_out = dramp.tile(
                [NCORES * (UU + EE), D], BF16, addr_space="Shared", name="cc_y_out"
            )
            cc_pois_in = dramp.tile([2 * PP, D], BF16, name="cc_pois_in")
            cc_pois_out = dramp.tile(
                [NCORES * 2 * PP, D], BF16, addr_space="Shared", name="cc_pois_out"
            )

            # ---- constants ----------------------------------------------
            sb_ident = constp.tile([D, D], F32, name="sb_ident")
            nc.sync.dma_start(sb_ident[:], ident)
            sb_w = constp.tile([D, 3, D], F32, name="sb_w")
            nc.sync.dma_start(sb_w[:], w3.rearrange("t k m -> k t m"))
            sb_bT = constp.tile([D, 3], F32, name="sb_bT")
            nc.sync.dma_start(sb_bT[:], bT3)
            sb_peT = constp.tile([D, PP], F32, name="sb_peT")
            nc.sync.dma_start(sb_peT[:], peT)

            # gate results, transposed layout [D, PP], f32 — kept resident
            sb_gateT = [
                constp.tile([D, PP], F32, name=f"sb_gateT{t}") for t in range(3)
            ]

            def nat_store(srcT, dram_rows, n_m):
                """PE-transpose [D, n_m*128] srcT (f32) into natural layout and
                DMA it (cast to bf16) into dram_rows ([n_m*128, D] DRAM view)."""
                nat = stagep.tile([128, n_m * 128], BF16, tag="nat")
                for m in range(n_m):
                    pst = pstr.tile([128, 128], F32, tag="tr")
                    nc.tensor.transpose(
                        pst[:], srcT[:, m * 128 : (m + 1) * 128], sb_ident[:]
                    )
                    nc.vector.tensor_copy(nat[:, m * 128 : (m + 1) * 128], pst[:])
                nc.sync.dma_start(
                    dram_rows.rearrange("(m p) d -> p m d", p=128),
                    nat[:].rearrange("p (m d) -> p m d", d=D),
                )

            # ---- phase A: gates (f32 / f32r matmuls) --------------------
            for t in range(3):
                psg = psacc.tile([D, PP], F32, tag="acc")
                for h in range(2):
                    cols = slice(512 * h, 512 * (h + 1))
                    nc.tensor.matmul(
                        psg[:, cols],
                        sb_w[:, t, :],
                        sb_peT[:, cols],
                        start=True,
                        stop=True,
                    )
                sb_sig = stagep.tile([D, PP], F32, tag="sig")
                for h in range(2):
                    cols = slice(512 * h, 512 * (h + 1))
                    nc.scalar.activation(
                        sb_sig[:, cols], psg[:, cols], SIG, bias=sb_bT[:, t : t + 1]
                    )
                nc.vector.tensor_mul(sb_gateT[t], sb_peT[:], sb_sig[:])
                nat_store(sb_gateT[t], cc_gates_in[t * PP : (t + 1) * PP, :], PP // 128)

            nc.gpsimd.collective_compute(
                "AllGather",
                BYPASS,
                replica_groups=RG,
                ins=[cc_gates_in[:].opt()],
                outs=[cc_gates_out[:].opt()],
            )

            def load_full(cc_out, t_off, rows, stride, name):
                """Gather rank-blocks of an all-gathered [*, D] tensor into a
                resident SBUF tile [128, (k d)] of natural k-tiles."""
                kt = rows // 128  # k-tiles per rank
                full = bigp.tile([128, NCORES * kt * 128], BF16, tag="big", name=name)
                for r in range(NCORES):
                    src = cc_out[r * stride + t_off : r * stride + t_off + rows, :]
                    eng = nc.sync if r % 2 == 0 else nc.scalar
                    eng.dma_start(
                        full[:, r * kt * 128 : (r + 1) * kt * 128].rearrange(
                            "p (k d) -> p k d", d=D
                        ),
                        src.rearrange("(k p) d -> p k d", p=128),
                    )
                return full

            x_full = load_full(cc_gates_out, 0, PP, 3 * PP, "x_full")
            s_full = load_full(cc_gates_out, PP, PP, 3 * PP, "s_full")
            g_full = load_full(cc_gates_out, 2 * PP, PP, 3 * PP, "g_full")

            def stream_product(lhs_full, matT, n_k, n_out, psum_tiles, extra=None):
                """psum[d, n_out] += sum_k lhs_full_tile_k.T @ matT[k-tile, :].

                matT is a DRAM [n_k*128, n_out] bf16 tensor; streamed in ~1MB
                chunks.  psum_tiles: list of (psum_ap, col_off) output banks of
                width 512.  extra: optional (lhs_full2, psum2) to fuse a second
                product sharing the same rhs stream.
                """
                n512 = n_out // 512
                ck = max(1, (1 << 20) // (n_out * 2 * 128))  # k-tiles per ~1MB chunk
                n_chunks = (n_k + ck - 1) // ck
                for c in range(n_chunks):
                    k0 = c * ck
                    kn = min(ck, n_k - k0)
                    chunk = rhsp.tile([128, ck, n_out], BF16, tag="rhs")
                    eng = nc.sync if c % 2 == 0 else nc.scalar
                    eng.dma_start(
                        chunk[:, :kn, :],
                        matT[k0 * 128 : (k0 + kn) * 128, :].rearrange(
                            "(a p) n -> p a n", p=128
                        ),
                    )
                    for kk in range(kn):
                        k = k0 + kk
                        lhs_tile = lhs_full[:, k * 128 : (k + 1) * 128]
                        for n in range(n512):
                            cols = slice(512 * n, 512 * (n + 1))
                            nc.tensor.matmul(
                                psum_tiles[n][:, :],
                                lhs_tile,
                                chunk[:, kk, cols],
                                start=(k == 0),
                                stop=(k == n_k - 1),
                            )
                        if extra is not None:
                            lhs2, ps2 = extra
                            nc.tensor.matmul(
                                ps2[:, :],
                                lhs2[:, k * 128 : (k + 1) * 128],
                                chunk[:, kk, 0:512],
                                start=(k == 0),
                                stop=(k == n_k - 1),
                            )

            # ---- phase B1: y_up = HG_up @ x  (shard over U rows) --------
            ps_yu = psacc.tile([D, 512], F32, tag="acc")
            stream_product(x_full, UpT, KP, UU, [ps_yu])
            yuT = stagep.tile([D, UU], F32, tag="ysb")
            nc.vector.tensor_copy(yuT[:], ps_yu[:])
            nat_store(yuT, cc_y_in[0:UU, :], UU // 128)

            # ---- phase B2: y_tar = HG_poi_tar @ s  (shard over E rows) --
            ps_yt = psacc.tile([D, 512], F32, tag="acc")
            stream_product(s_full, TarT, KP, EE, [ps_yt])
            ytT = stagep.tile([D, EE], F32, tag="ysb")
            nc.vector.tensor_copy(ytT[:], ps_yt[:])
            nat_store(ytT, cc_y_in[UU : UU + EE, :], EE // 128)

            nc.gpsimd.collective_compute(
                "AllGather",
                BYPASS,
                replica_groups=RG,
                ins=[cc_y_in[:].opt()],
                outs=[cc_y_out[:].opt()],
            )

            # ---- phase B3: geo_pois = g + 0.4 * Geo @ g  (P-row shard) --
            ps_geo = psacc.tile([D, PP], F32, tag="acc")
            stream_product(
                g_full, GeoT, KP, PP, [ps_geo[:, 0:512], ps_geo[:, 512:1024]]
            )
            geo_poisT = outp.tile([D, PP], F32, tag="out", name="geo_poisT")
            nc.vector.scalar_tensor_tensor(
                geo_poisT[:], ps_geo[:], GEO_SCALE, sb_gateT[2][:], MULT, ADD
            )
            nc.sync.dma_start(poisT_o[1], geo_poisT[:])
            nat_store(geo_poisT, cc_pois_in[PP : 2 * PP, :], PP // 128)

            # ---- gathered y tensors -------------------------------------
            yup_full = load_full(cc_y_out, 0, UU, UU + EE, "yup_full")
            ytar_full = load_full(cc_y_out, UU, EE, UU + EE, "ytar_full")

            # ---- phase C1: hg_pois = x + HG_pu @ y_up  (P-row shard) ----
            ps_hg = psacc.tile([D, PP], F32, tag="acc")
            stream_product(
                yup_full, PuT, KU, PP, [ps_hg[:, 0:512], ps_hg[:, 512:1024]]
            )
            hg_poisT = outp.tile([D, PP], F32, tag="out", name="hg_poisT")
            nc.vector.tensor_add(hg_poisT[:], ps_hg[:], sb_gateT[0][:])
            nc.sync.dma_start(poisT_o[0], hg_poisT[:])
            nat_store(hg_poisT, cc_pois_in[0:PP, :], PP // 128)

            nc.gpsimd.collective_compute(
                "AllGather",
                BYPASS,
                replica_groups=RG,
                ins=[cc_pois_in[:].opt()],
                outs=[cc_pois_out[:].opt()],
            )

            # ---- phase C2: trans_pois = s + Src @ y_tar  (P-row shard) --
            ps_tr = psacc.tile([D, PP], F32, tag="acc")
            stream_product(
                ytar_full, SrcT, KU, PP, [ps_tr[:, 0:512], ps_tr[:, 512:1024]]
            )
            trans_poisT = outp.tile([D, PP], F32, tag="out", name="trans_poisT")
            nc.vector.tensor_add(trans_poisT[:], ps_tr[:], sb_gateT[1][:])
            nc.sync.dma_start(poisT_o[2], trans_poisT[:])

            # ---- gathered pois ------------------------------------------
            hg_full = load_full(cc_pois_out, 0, PP, 2 * PP, "hg_full")
            geo_full = load_full(cc_pois_out, PP, PP, 2 * PP, "geo_full")

            # ---- phase D: user embeddings (shard over U rows) -----------
            ps_hu = psacc.tile([D, 512], F32, tag="acc")
            ps_gu = psacc.tile([D, 512], F32, tag="acc")
            stream_product(hg_full, UpT, KP, UU, [ps_hu], extra=(geo_full, ps_gu))
            users_sb = outp.tile([D, 2 * UU], F32, tag="out", name="users_sb")
            nc.vector.tensor_copy(users_sb[:, 0:UU], ps_hu[:])
            nc.vector.tensor_copy(users_sb[:, UU : 2 * UU], ps_gu[:])
            nc.sync.dma_start(usersT_o, users_sb[:])

    nc.compile()
    return nc


def _get_nc():
    if "nc" not in _CACHE:
        _CACHE["nc"] = _build_nc()
    return _CACHE["nc"]


def _shard_inputs(inputs):
    f32 = np.float32
    bf16 = ml_dtypes.bfloat16
    pe = np.asarray(inputs["poi_emb_weight"], f32)[:P]
    w3 = np.stack(
        [
            np.asarray(inputs["w_gate_col"], f32),
            np.asarray(inputs["w_gate_seq"], f32),
            np.asarray(inputs["w_gate_geo"], f32),
        ]
    )
    bT3 = np.stack(
        [
            np.asarray(inputs["b_gate_col"], f32)[0],
            np.asarray(inputs["b_gate_seq"], f32)[0],
            np.asarray(inputs["b_gate_geo"], f32)[0],
        ],
        axis=1,
    )
    eye = np.eye(D, dtype=f32)
    Up = np.asarray(inputs["HG_up"], f32)
    Pu = np.asarray(inputs["HG_pu"], f32)
    Tar = np.asarray(inputs["HG_poi_tar"], f32)
    Src = np.asarray(inputs["HG_poi_src"], f32)
    Geo = np.asarray(inputs["poi_geo_graph"], f32)

    in_maps = []
    for i in range(NCORES):
        rp = slice(PP * i, PP * (i + 1))
        ru = slice(UU * i, UU * (i + 1))
        re_ = slice(EE * i, EE * (i + 1))
        in_maps.append(
            {
                "peT": np.ascontiguousarray(pe[rp].T),
                "w3": w3,
                "bT3": bT3,
                "ident": eye,
                "UpT": np.ascontiguousarray(Up[ru].T).astype(bf16),
                "PuT": np.ascontiguousarray(Pu[rp].T).astype(bf16),
                "TarT": np.ascontiguousarray(Tar[re_].T).astype(bf16),
                "SrcT": np.ascontiguousarray(Src[rp].T).astype(bf16),
                "GeoT": np.ascontiguousarray(Geo[rp].T).astype(bf16),
            }
        )
    return in_maps


def _assemble(results, user_idx):
    f32 = np.float32
    hg = np.empty((P, D), f32)
    geo = np.empty((P, D), f32)
    tr = np.empty((P, D), f32)
    hgu = np.empty((U, D), f32)
    geou = np.empty((U, D), f32)
    for i in range(NCORES):
        rp = slice(PP * i, PP * (i + 1))
        ru = slice(UU * i, UU * (i + 1))
        pois = results[i]["poisT_o"]
        hg[rp] = pois[0].T
        geo[rp] = pois[1].T
        tr[rp] = pois[2].T
        users = results[i]["usersT_o"]
        hgu[ru] = users[:, :UU].T
        geou[ru] = users[:, UU:].T
    idx = np.asarray(user_idx)
    return np.concatenate([hg, geo, tr, hgu[idx], geou[idx]], axis=0)


def _run(inputs, trace=False, **spmd_kwargs):
    nc = _get_nc()
    in_maps = _shard_inputs(inputs)
    res = run_bass_kernel_spmd(
        nc, in_maps, list(range(NCORES)), trace=trace, **spmd_kwargs
    )
    return _assemble(res.results, inputs["user_idx"]), res


def kernel(**inputs):
    return _run(inputs)[0]


if __name__ == "__main__":
    import pickle

    with open("/tmp/inputs.pkl", "rb") as f:
        inputs = pickle.load(f)
    out = kernel(**inputs)
    exp = np.load("/tmp/expected.npy")
    rel = np.linalg.norm(out - exp) / np.linalg.norm(exp)
    print("Relative error:", rel)


# revision 8
# speedup vs baseline: 1.0861x; 1.0861x over previous
"""Trainium2 Bass kernel for nn_HODE_MDP (hypergraph ODE message passing).

Math (T_UP = T_GEO = T_P2P = 1.0, ALPHA = 0.8):
    pe  = poi_emb_weight[:-1]                      # [P, D]
    x/s/g = pe * sigmoid(pe @ W_t + b_t)           # col / seq / geo gates
    hg_pois    = x + HG_pu @ (HG_up @ x)
    geo_pois   = g + 0.4 * (poi_geo_graph @ g)
    trans_pois = s + HG_poi_src @ (HG_poi_tar @ s)
    hg_users   = (HG_up @ hg_pois)[user_idx]
    geo_users  = (HG_up @ geo_pois)[user_idx]
    out = concat([hg_pois, geo_pois, trans_pois, hg_users, geo_users])

Distribution (8 NeuronCores): shard the row dim of every big matrix
(P rows for HG_pu / HG_poi_src / poi_geo_graph, U rows for HG_up, E rows
for HG_poi_tar).  Each core computes its row-block of each product with
the full activation (all-gathered [*, D] tensors).  Matrix blocks are
shipped pre-transposed ([contract_dim, out_block]) so the PE consumes
them as the moving operand with N=512; the activation k-tile [128, 128]
is the stationary operand.  All products run in bf16 (f32 PSUM accum);
gates stay f32 — the ODE deltas are ~1e-5 of the output scale so bf16
matmul error is negligible (measured l2 rel err 6.6e-6 on the full
pipeline vs f32).

Outputs come back transposed [D, block]; the host re-transposes,
concatenates, and applies the user_idx gather.
"""

import sys

if "/opt/trn_rl_repo" not in sys.path:
    sys.path.insert(0, "/opt/trn_rl_repo")

import numpy as np
import ml_dtypes

import concourse.bass as bass  # noqa: F401  (engine types via nc)
import concourse.bacc as bacc
import concourse.mybir as mybir
import concourse.tile as tile
from concourse.bass_utils import run_bass_kernel_spmd

F32 = mybir.dt.float32
BF16 = mybir.dt.bfloat16
SIG = mybir.ActivationFunctionType.Sigmoid
MULT = mybir.AluOpType.mult
ADD = mybir.AluOpType.add
BYPASS = mybir.AluOpType.bypass

NCORES = 8
P, U, E, D = 8192, 4096, 4096, 128
PP, UU, EE = P // NCORES, U // NCORES, E // NCORES  # 1024, 512, 512
KP, KU = P // 128, U // 128                         # 64, 32 k-tiles
RG = [list(range(NCORES))]
GEO_SCALE = 0.4  # ALPHA / 2 * T_GEO

_CACHE: dict = {}


def _build_nc():
    nc = bacc.Bacc(
        "TRN2",
        target_bir_lowering=False,
        debug=False,
        enable_asserts=False,
        num_devices=NCORES,
    )

    # ---- per-core DRAM I/O ----------------------------------------------
    peT = nc.dram_tensor("peT", [D, PP], F32, kind="ExternalInput").ap()
    w3 = nc.dram_tensor("w3", [3, D, D], F32, kind="ExternalInput").ap()
    bT3 = nc.dram_tensor("bT3", [D, 3], F32, kind="ExternalInput").ap()
    ident = nc.dram_tensor("ident", [D, D], F32, kind="ExternalInput").ap()
    UpT = nc.dram_tensor("UpT", [P, UU], BF16, kind="ExternalInput").ap()
    PuT = nc.dram_tensor("PuT", [U, PP], BF16, kind="ExternalInput").ap()
    TarT = nc.dram_tensor("TarT", [P, EE], BF16, kind="ExternalInput").ap()
    SrcT = nc.dram_tensor("SrcT", [E, PP], BF16, kind="ExternalInput").ap()
    GeoT = nc.dram_tensor("GeoT", [P, PP], BF16, kind="ExternalInput").ap()

    poisT_o = nc.dram_tensor("poisT_o", [3, D, PP], F32, kind="ExternalOutput").ap()
    usersT_o = nc.dram_tensor("usersT_o", [D, 2 * UU], F32, kind="ExternalOutput").ap()

    with tile.TileContext(nc) as tc:
        with (
            tc.tile_pool(name="const", bufs=1) as constp,
            tc.tile_pool(name="big", bufs=3) as bigp,
            tc.tile_pool(name="rhs", bufs=10) as rhsp,
            tc.tile_pool(name="stage", bufs=2) as stagep,
            tc.tile_pool(name="outp", bufs=2) as outp,
            tc.tile_pool(name="psacc", bufs=2, space="PSUM") as psacc,
            tc.tile_pool(name="pstr", bufs=2, space="PSUM") as pstr,
            tc.tile_pool(name="dram", bufs=1, space="DRAM") as dramp,
        ):
            # ---- internal DRAM collective buffers -----------------------
            # SBUF-mirror layout: [128, cols] where cols = (k d) natural
            # k-tiles side by side -> every bounce DMA moves contiguous
            # >=1KB per-partition lines, and AllGather concatenation along
            # axis 0 stacks rank blocks [128r:128r+128, :].
            cc_x_in = dramp.tile([128, PP], BF16, name="cc_x_in")
            cc_x_out = dramp.tile(
                [NCORES * 128, PP], BF16, addr_space="Shared", name="cc_x_out"
            )
            cc_sg_in = dramp.tile([128, 2 * PP], BF16, name="cc_sg_in")
            cc_sg_out = dramp.tile(
                [NCORES * 128, 2 * PP], BF16, addr_space="Shared", name="cc_sg_out"
            )
            cc_y_in = dramp.tile([128, UU + EE], BF16, name="cc_y_in")
            cc_y_out = dramp.tile(
                [NCORES * 128, UU + EE], BF16, addr_space="Shared", name="cc_y_out"
            )
            cc_pois_in = dramp.tile([128, 2 * PP], BF16, name="cc_pois_in")
            cc_pois_out = dramp.tile(
                [NCORES * 128, 2 * PP], BF16, addr_space="Shared", name="cc_pois_out"
            )

            # ---- constants (gpsimd: keep sync/scalar queues pure stream) -
            sb_ident = constp.tile([D, D], F32, name="sb_ident")
            nc.gpsimd.dma_start(sb_ident[:], ident)
            sb_w = constp.tile([D, 3, D], F32, name="sb_w")
            nc.gpsimd.dma_start(sb_w[:], w3.rearrange("t k m -> k t m"))
            sb_bT = constp.tile([D, 3], F32, name="sb_bT")
            nc.gpsimd.dma_start(sb_bT[:], bT3)
            sb_peT = constp.tile([D, PP], F32, name="sb_peT")
            nc.gpsimd.dma_start(sb_peT[:], peT)

            # gate results, transposed layout [D, PP], f32 — kept resident
            sb_gateT = [
                constp.tile([D, PP], F32, name=f"sb_gateT{t}") for t in range(3)
            ]

            def nat_store(srcT, cc_dst, n_m):
                """PE-transpose [D, n_m*128] srcT (f32) into natural k-tiles
                and DMA (cast to bf16) into cc_dst ([128, n_m*128] DRAM)."""
                nat = stagep.tile([128, n_m * 128], BF16, tag="nat")
                for m in range(n_m):
                    pst = pstr.tile([128, 128], F32, tag="tr")
                    nc.tensor.transpose(
                        pst[:], srcT[:, m * 128 : (m + 1) * 128], sb_ident[:]
                    )
                    nc.vector.tensor_copy(nat[:, m * 128 : (m + 1) * 128], pst[:])
                nc.gpsimd.dma_start(cc_dst, nat[:])

            # ---- phase A: gates (fp32 matmuls, tiny) --------------------
            def gate(t):
                psg = psacc.tile([D, PP], F32, tag="acc")
                for h in range(2):
                    cols = slice(512 * h, 512 * (h + 1))
                    nc.tensor.matmul(
                        psg[:, cols],
                        sb_w[:, t, :],
                        sb_peT[:, cols],
                        start=True,
                        stop=True,
                    )
                sb_sig = stagep.tile([D, PP], F32, tag="sig")
                for h in range(2):
                    cols = slice(512 * h, 512 * (h + 1))
                    nc.scalar.activation(
                        sb_sig[:, cols], psg[:, cols], SIG, bias=sb_bT[:, t : t + 1]
                    )
                nc.vector.tensor_mul(sb_gateT[t], sb_peT[:], sb_sig[:])

            # x (col gate) first so its AllGather fires as early as possible
            gate(0)
            nat_store(sb_gateT[0], cc_x_in[:, :], PP // 128)
            nc.gpsimd.collective_compute(
                "AllGather",
                BYPASS,
                replica_groups=RG,
                ins=[cc_x_in[:].opt()],
                outs=[cc_x_out[:].opt()],
            )
            gate(1)
            nat_store(sb_gateT[1], cc_sg_in[:, 0:PP], PP // 128)
            gate(2)
            nat_store(sb_gateT[2], cc_sg_in[:, PP : 2 * PP], PP // 128)
            nc.gpsimd.collective_compute(
                "AllGather",
                BYPASS,
                replica_groups=RG,
                ins=[cc_sg_in[:].opt()],
                outs=[cc_sg_out[:].opt()],
            )

            def load_full(cc_out, col0, cols, name):
                """Gather rank blocks [128r:128r+128, col0:col0+cols] of an
                all-gathered SBUF-mirror tensor into one SBUF tile of natural
                k-tiles (contiguous per-partition lines)."""
                full = bigp.tile([128, NCORES * cols], BF16, tag="big", name=name)
                for r in range(NCORES):
                    nc.gpsimd.dma_start(
                        full[:, r * cols : (r + 1) * cols],
                        cc_out[r * 128 : (r + 1) * 128, col0 : col0 + cols],
                    )
                return full

            x_full = load_full(cc_x_out, 0, PP, "x_full")
            s_full = load_full(cc_sg_out, 0, PP, "s_full")
            g_full = load_full(cc_sg_out, PP, PP, "g_full")

            def stream_product(lhs_full, matT, n_k, n_out, psum_tiles, extra=None):
                """psum[d, n_out] += sum_k lhs_full_tile_k.T @ matT[k-tile, :].

                matT is a DRAM [n_k*128, n_out] bf16 tensor; streamed in ~1MB
                chunks.  psum_tiles: list of (psum_ap, col_off) output banks of
                width 512.  extra: optional (lhs_full2, psum2) to fuse a second
                product sharing the same rhs stream.
                """
                n512 = n_out // 512
                ck = max(1, (1 << 20) // (n_out * 2 * 128))  # k-tiles per ~1MB chunk
                n_chunks = (n_k + ck - 1) // ck
                for c in range(n_chunks):
                    k0 = c * ck
                    kn = min(ck, n_k - k0)
                    chunk = rhsp.tile([128, ck, n_out], BF16, tag="rhs")
                    eng = nc.sync if c % 2 == 0 else nc.scalar
                    eng.dma_start(
                        chunk[:, :kn, :],
                        matT[k0 * 128 : (k0 + kn) * 128, :].rearrange(
                            "(a p) n -> p a n", p=128
                        ),
                    )
                    for kk in range(kn):
                        k = k0 + kk
                        lhs_tile = lhs_full[:, k * 128 : (k + 1) * 128]
                        for n in range(n512):
                            cols = slice(512 * n, 512 * (n + 1))
                            nc.tensor.matmul(
                                psum_tiles[n][:, :],
                                lhs_tile,
                                chunk[:, kk, cols],
                                start=(k == 0),
                                stop=(k == n_k - 1),
                            )
                        if extra is not None:
                            lhs2, ps2 = extra
                            nc.tensor.matmul(
                                ps2[:, :],
                                lhs2[:, k * 128 : (k + 1) * 128],
                                chunk[:, kk, 0:512],
                                start=(k == 0),
                                stop=(k == n_k - 1),
                            )

            # ---- phase B1: y_up = HG_up @ x  (shard over U rows) --------
            ps_yu = psacc.tile([D, 512], F32, tag="acc")
            stream_product(x_full, UpT, KP, UU, [ps_yu])
            yuT = stagep.tile([D, UU], F32, tag="ysb")
            nc.vector.tensor_copy(yuT[:], ps_yu[:])
            nat_store(yuT, cc_y_in[:, 0:UU], UU // 128)

            # ---- phase B2: y_tar = HG_poi_tar @ s  (shard over E rows) --
            ps_yt = psacc.tile([D, 512], F32, tag="acc")
            stream_product(s_full, TarT, KP, EE, [ps_yt])
            ytT = stagep.tile([D, EE], F32, tag="ysb")
            nc.vector.tensor_copy(ytT[:], ps_yt[:])
            nat_store(ytT, cc_y_in[:, UU : UU + EE], EE // 128)

            nc.gpsimd.collective_compute(
                "AllGather",
                BYPASS,
                replica_groups=RG,
                ins=[cc_y_in[:].opt()],
                outs=[cc_y_out[:].opt()],
            )

            # ---- phase B3: geo_pois = g + 0.4 * Geo @ g  (P-row shard) --
            ps_geo = psacc.tile([D, PP], F32, tag="acc")
            stream_product(
                g_full, GeoT, KP, PP, [ps_geo[:, 0:512], ps_geo[:, 512:1024]]
            )
            geo_poisT = outp.tile([D, PP], F32, tag="out", name="geo_poisT")
            nc.vector.scalar_tensor_tensor(
                geo_poisT[:], ps_geo[:], GEO_SCALE, sb_gateT[2][:], MULT, ADD
            )
            nc.gpsimd.dma_start(poisT_o[1], geo_poisT[:])
            nat_store(geo_poisT, cc_pois_in[:, PP : 2 * PP], PP // 128)

            # ---- gathered y tensors -------------------------------------
            yup_full = load_full(cc_y_out, 0, UU, "yup_full")
            ytar_full = load_full(cc_y_out, UU, EE, "ytar_full")

            # ---- phase C1: hg_pois = x + HG_pu @ y_up  (P-row shard) ----
            ps_hg = psacc.tile([D, PP], F32, tag="acc")
            stream_product(
                yup_full, PuT, KU, PP, [ps_hg[:, 0:512], ps_hg[:, 512:1024]]
            )
            hg_poisT = outp.tile([D, PP], F32, tag="out", name="hg_poisT")
            nc.vector.tensor_add(hg_poisT[:], ps_hg[:], sb_gateT[0][:])
            nc.gpsimd.dma_start(poisT_o[0], hg_poisT[:])
            nat_store(hg_poisT, cc_pois_in[:, 0:PP], PP // 128)

            nc.gpsimd.collective_compute(
                "AllGather",
                BYPASS,
                replica_groups=RG,
                ins=[cc_pois_in[:].opt()],
                outs=[cc_pois_out[:].opt()],
            )

            # ---- phase C2: trans_pois = s + Src @ y_tar  (P-row shard) --
            ps_tr = psacc.tile([D, PP], F32, tag="acc")
            stream_product(
                ytar_full, SrcT, KU, PP, [ps_tr[:, 0:512], ps_tr[:, 512:1024]]
            )
            trans_poisT = outp.tile([D, PP], F32, tag="out", name="trans_poisT")
            nc.vector.tensor_add(trans_poisT[:], ps_tr[:], sb_gateT[1][:])
            nc.gpsimd.dma_start(poisT_o[2], trans_poisT[:])

            # ---- gathered pois ------------------------------------------
            hg_full = load_full(cc_pois_out, 0, PP, "hg_full")
            geo_full = load_full(cc_pois_out, PP, PP, "geo_full")

            # ---- phase D: user embeddings (shard over U rows) -----------
            ps_hu = psacc.tile([D, 512], F32, tag="acc")
            ps_gu = psacc.tile([D, 512], F32, tag="acc")
            stream_product(hg_full, UpT, KP, UU, [ps_hu], extra=(geo_full, ps_gu))
            users_sb = outp.tile([D, 2 * UU], F32, tag="out", name="users_sb")
            nc.vector.tensor_copy(users_sb[:, 0:UU], ps_hu[:])
            nc.vector.tensor_copy(users_sb[:, UU : 2 * UU], ps_gu[:])
            nc.gpsimd.dma_start(usersT_o, users_sb[:])

    nc.compile()
    return nc


def _get_nc():
    if "nc" not in _CACHE:
        _CACHE["nc"] = _build_nc()
    return _CACHE["nc"]


def _shard_inputs(inputs):
    f32 = np.float32
    bf16 = ml_dtypes.bfloat16
    pe = np.asarray(inputs["poi_emb_weight"], f32)[:P]
    w3 = np.stack(
        [
            np.asarray(inputs["w_gate_col"], f32),
            np.asarray(inputs["w_gate_seq"], f32),
            np.asarray(inputs["w_gate_geo"], f32),
        ]
    )
    bT3 = np.stack(
        [
            np.asarray(inputs["b_gate_col"], f32)[0],
            np.asarray(inputs["b_gate_seq"], f32)[0],
            np.asarray(inputs["b_gate_geo"], f32)[0],
        ],
        axis=1,
    )
    eye = np.eye(D, dtype=f32)
    Up = np.asarray(inputs["HG_up"], f32)
    Pu = np.asarray(inputs["HG_pu"], f32)
    Tar = np.asarray(inputs["HG_poi_tar"], f32)
    Src = np.asarray(inputs["HG_poi_src"], f32)
    Geo = np.asarray(inputs["poi_geo_graph"], f32)

    in_maps = []
    for i in range(NCORES):
        rp = slice(PP * i, PP * (i + 1))
        ru = slice(UU * i, UU * (i + 1))
        re_ = slice(EE * i, EE * (i + 1))
        in_maps.append(
            {
                "peT": np.ascontiguousarray(pe[rp].T),
                "w3": w3,
                "bT3": bT3,
                "ident": eye,
                "UpT": np.ascontiguousarray(Up[ru].T).astype(bf16),
                "PuT": np.ascontiguousarray(Pu[rp].T).astype(bf16),
                "TarT": np.ascontiguousarray(Tar[re_].T).astype(bf16),
                "SrcT": np.ascontiguousarray(Src[rp].T).astype(bf16),
                "GeoT": np.ascontiguousarray(Geo[rp].T).astype(bf16),
            }
        )
    return in_maps


def _assemble(results, user_idx):
    f32 = np.float32
    hg = np.empty((P, D), f32)
    geo = np.empty((P, D), f32)
    tr = np.empty((P, D), f32)
    hgu = np.empty((U, D), f32)
    geou = np.empty((U, D), f32)
    for i in range(NCORES):
        rp = slice(PP * i, PP * (i + 1))
        ru = slice(UU * i, UU * (i + 1))
        pois = results[i]["poisT_o"]
        hg[rp] = pois[0].T
        geo[rp] = pois[1].T
        tr[rp] = pois[2].T
        users = results[i]["usersT_o"]
        hgu[ru] = users[:, :UU].T
        geou[ru] = users[:, UU:].T
    idx = np.asarray(user_idx)
    return np.concatenate([hg, geo, tr, hgu[idx], geou[idx]], axis=0)


def _run(inputs, trace=False, **spmd_kwargs):
    nc = _get_nc()
    in_maps = _shard_inputs(inputs)
    res = run_bass_kernel_spmd(
        nc, in_maps, list(range(NCORES)), trace=trace, **spmd_kwargs
    )
    return _assemble(res.results, inputs["user_idx"]), res


def kernel(**inputs):
    return _run(inputs)[0]


if __name__ == "__main__":
    import pickle

    with open("/tmp/inputs.pkl", "rb") as f:
        inputs = pickle.load(f)
    out = kernel(**inputs)
    exp = np.load("/tmp/expected.npy")
    rel = np.linalg.norm(out - exp) / np.linalg.norm(exp)
    print("Relative error:", rel)
